# revision 18
# baseline (speedup 1.0000x reference)
"""Trainium2 Bass kernel for BertLinearSelfAttention (linear attention).

Reference computation (per batch b, head h):
    q,k,v = X @ W{q,k,v} + b{q,k,v}            # [S, D] -> heads of 64
    qf, kf = elu(q)+1, elu(k)+1                # = min(exp(x),1) + max(x,0)
    kv[d,e]  = sum_s kf[s,d] v[s,e]            # [64, 64]
    ksum[d]  = sum_s kf[s,d]
    out[s,e] = (sum_d qf[s,d] kv[d,e]) / (sum_d qf[s,d] ksum[d])

Sharding: 8 cores = (4 batches) x (2 head-groups of 8 heads / 512 proj cols).
All matmul operands are bf16 (converted host-side), which keeps every matmul
at the PE's 1 row/cycle stream rate: fp32 operands are SBUF-read-bandwidth
bound (~0.92 ns/row measured vs 0.42 compute), bf16 halves the traffic.

Pass A (per 512-token chunk): k/v projections (tokens on partitions) +
feature maps on DVE/ACT + per-head kv/ksum accumulated directly in PSUM
across all chunks (ones-column in V' produces ksum).
Pass B (per chunk): q^T projection (cols on partitions), then per head-pair
block-diagonal den/num matmuls with N=512 moving tokens:
    den^T[p,s] = sum_k ksumrep[k,p] qf^T[k,s]   (ksum replicated across the
                 64 e-columns of its head, so the PE broadcasts den for free)
    num^T[e,s] = sum_d kv[d,e] qf^T[d,s]
Divide runs on DVE as the PSUM evict (recip + mult), output is stored
transposed [cols, tokens] in bf16 and re-transposed/upcast on the host.
"""

import os
import sys

import numpy as np

_REPO = "/opt/trn_rl_repo"
if os.path.isdir(_REPO) and _REPO not in sys.path:
    sys.path.insert(0, _REPO)

B, S, D, H, HD = 4, 4096, 1024, 16, 64
NCORES = 8
CG = 512            # projection columns per core (8 heads)
NH = CG // HD       # 8 heads per core
NCT = CG // 128     # 4 head-pair column tiles
HE = HD + 2         # vp cols per head: 64 v + 1 ones (ksum) + 1 pad
CHUNK = 512         # tokens per chunk
NSUB = CHUNK // 128     # 4 token sub-tiles per chunk
NCHUNK = S // CHUNK     # 8 chunks
NKT = D // 128          # 8 contraction tiles
P = 128

_CACHED_NC = None


def _build():
    import concourse.tile as tile
    from concourse import bacc, mybir
    from contextlib import ExitStack

    F32 = mybir.dt.float32
    BF16 = mybir.dt.bfloat16
    Alu = mybir.AluOpType
    Act = mybir.ActivationFunctionType

    nc = bacc.Bacc("TRN2", target_bir_lowering=False, debug=False,
                   num_devices=NCORES)

    xt_d = nc.dram_tensor("xt", [D, S], BF16, kind="ExternalInput").ap()
    w_d = {
        "q": nc.dram_tensor("wq", [D, CG], BF16, kind="ExternalInput").ap(),
        "k": nc.dram_tensor("wk", [D, CG], BF16, kind="ExternalInput").ap(),
        "v": nc.dram_tensor("wv", [D, CG], BF16, kind="ExternalInput").ap(),
    }
    bq_d = nc.dram_tensor("bq", [P, NCT], F32, kind="ExternalInput").ap()
    # bk/bv pre-replicated across partitions host-side (keeps gpsimd as a
    # pure DMA queue — its compute path costs a ~12us LOAD_LIB+DRAIN)
    bk_d = nc.dram_tensor("bk", [P, CG], F32, kind="ExternalInput").ap()
    bv_d = nc.dram_tensor("bv", [P, CG], F32, kind="ExternalInput").ap()
    out_d = nc.dram_tensor("out", [S, CG], BF16, kind="ExternalOutput").ap()

    with tile.TileContext(nc) as tc:
        with ExitStack() as ctx:
            const = ctx.enter_context(tc.tile_pool(name="const", bufs=1))
            wpool = ctx.enter_context(tc.tile_pool(name="wpool", bufs=1))
            xtpool = ctx.enter_context(tc.tile_pool(name="xtpool", bufs=24))
            kfpool = ctx.enter_context(tc.tile_pool(name="kfpool", bufs=8))
            vppool = ctx.enter_context(tc.tile_pool(name="vppool", bufs=8))
            qftpool = ctx.enter_context(tc.tile_pool(name="qftpool", bufs=8))
            tmp = ctx.enter_context(tc.tile_pool(name="tmp", bufs=10))
            outpool = ctx.enter_context(tc.tile_pool(name="outp", bufs=6))
            recpool = ctx.enter_context(tc.tile_pool(name="recp", bufs=8))
            kvbpool = ctx.enter_context(tc.tile_pool(name="kvbp", bufs=1))
            pps = ctx.enter_context(
                tc.tile_pool(name="pps", bufs=2, space="PSUM"))
            kvps = ctx.enter_context(
                tc.tile_pool(name="kvps", bufs=1, space="PSUM"))
            dnps = ctx.enter_context(
                tc.tile_pool(name="dnps", bufs=2, space="PSUM"))

            def load_xt(ci):
                tok0 = ci * CHUNK
                xt = []
                for kt in range(NKT):
                    t = xtpool.tile([P, CHUNK], BF16, tag="xt", name="xt")
                    nc.sync.dma_start(
                        t[:], xt_d[kt * P:(kt + 1) * P, tok0:tok0 + CHUNK])
                    xt.append(t)
                return xt

            # queue the first chunk's X^T ahead of all setup DMAs
            xt0 = load_xt(0)

            # ---- weights (per-kt tiles so the first matmul only waits on
            # its own 128KB slice), k/v interleaved first, q later ----
            w_t = {"q": [], "k": [], "v": []}
            for kt in range(NKT):
                for nm in ("k", "v"):
                    t = wpool.tile([P, CG], BF16, tag=f"w{nm}{kt}",
                                   name=f"w{nm}{kt}")
                    nc.gpsimd.dma_start(t[:], w_d[nm][kt * P:(kt + 1) * P, :])
                    w_t[nm].append(t)
            for kt in range(NKT):
                t = wpool.tile([P, CG], BF16, tag=f"wq{kt}", name=f"wq{kt}")
                nc.gpsimd.dma_start(t[:], w_d["q"][kt * P:(kt + 1) * P, :])
                w_t["q"].append(t)

            # ---- constants ----
            bq_sb = const.tile([P, NCT], F32, tag="bq", name="bq_sb")
            nc.gpsimd.dma_start(bq_sb[:], bq_d[:])
            bk_rep = const.tile([P, CG], F32, tag="bkrep", name="bk_rep")
            nc.sync.dma_start(bk_rep[:], bk_d[:])
            bv_rep = const.tile([P, CG], F32, tag="bvrep", name="bv_rep")
            nc.sync.dma_start(bv_rep[:], bv_d[:])
            # tail columns for V': [1.0 (ksum), 0.0 (pad)] per head
            ones_tail = const.tile([P, NH * 2], BF16, tag="otail",
                                   name="ones_tail")
            nc.vector.memset(ones_tail[:], 0.0)
            nc.vector.memset(
                ones_tail[:].rearrange("p (h e) -> p h e", e=2)[:, :, 0:1],
                1.0)

            # kv/ksum accumulators: bank i holds ct=2i (cols 0:132) and
            # ct=2i+1 (cols 132:264); within a ct: even head on partitions
            # 0:64 cols 0:66, odd head on partitions 64:128 cols 66:132.
            # NOTE: matmul start=True zeroes the full bank width for the
            # partitions it writes, so concurrent accumulation groups in one
            # bank must NOT use start; memset once and accumulate throughout.
            kvacc = [kvps.tile([P, 4 * HE], F32, tag=f"kvacc{i}",
                               name=f"kvacc{i}") for i in range(2)]
            for i in range(2):
                nc.vector.memset(kvacc[i][:], 0.0)

            kf_c = {}
            vp_c = {}
            qft_c = {}

            def a_chunk(ci, xt):
                kfs, vps = [], []
                for sub in range(NSUB):
                    sl = slice(sub * P, (sub + 1) * P)
                    kps = pps.tile([P, CG], F32, tag="pps", name="kps")
                    for kt in range(NKT):
                        nc.tensor.matmul(
                            kps[:], xt[kt][:, sl], w_t["k"][kt][:],
                            start=(kt == 0), stop=(kt == NKT - 1))
                    vps_ = pps.tile([P, CG], F32, tag="pps", name="vps")
                    for kt in range(NKT):
                        nc.tensor.matmul(
                            vps_[:], xt[kt][:, sl], w_t["v"][kt][:],
                            start=(kt == 0), stop=(kt == NKT - 1))
                    # k feature map: kf = min(exp(k+bk),1) + max(k+bk,0)
                    kb = tmp.tile([P, CG], BF16, tag="tmp", name="kb")
                    nc.vector.tensor_tensor(kb[:], kps[:], bk_rep[:], Alu.add)
                    e = tmp.tile([P, CG], BF16, tag="tmp", name="e")
                    nc.scalar.activation(e[:], kb[:], Act.Exp)
                    r = tmp.tile([P, CG], BF16, tag="tmp", name="r")
                    nc.scalar.activation(r[:], kb[:], Act.Relu)
                    m = tmp.tile([P, CG], BF16, tag="tmp", name="m")
                    nc.vector.tensor_scalar(m[:], e[:], 1.0, None, Alu.min)
                    kf = kfpool.tile([P, CG], BF16, tag="kf", name="kf")
                    nc.vector.tensor_tensor(kf[:], m[:], r[:], Alu.add)
                    kfs.append(kf)
                    # V' = [v + bv | 1 | 0] per head
                    vp = vppool.tile([P, NH * HE], BF16, tag="vp", name="vp")
                    nc.vector.tensor_tensor(
                        vp[:].rearrange("p (h e) -> p h e", e=HE)[:, :, :HD],
                        vps_[:].rearrange("p (h e) -> p h e", e=HD),
                        bv_rep[:].rearrange("p (h e) -> p h e", e=HD),
                        Alu.add)
                    nc.vector.tensor_copy(
                        vp[:].rearrange("p (h e) -> p h e", e=HE)[:, :, HD:],
                        ones_tail[:].rearrange("p (h e) -> p h e", e=2))
                    vps.append(vp)
                kf_c[ci] = kfs
                vp_c[ci] = vps

            def a_kv(ci):
                kfs, vps = kf_c.pop(ci), vp_c.pop(ci)
                for ct in range(NCT):
                    bank = kvacc[ct // 2]
                    base = (ct % 2) * 2 * HE
                    for par in range(2):
                        h = 2 * ct + par
                        dst = bank[par * HD:(par + 1) * HD,
                                   base + par * HE:base + (par + 1) * HE]
                        for sub in range(NSUB):
                            nc.tensor.matmul(
                                dst,
                                kfs[sub][:, h * HD:(h + 1) * HD],
                                vps[sub][:, h * HE:(h + 1) * HE],
                                start=False,
                                stop=(ci == NCHUNK - 1 and sub == NSUB - 1),
                                skip_group_check=True)

            kvb = []
            ksr = []

            def bridge():
                """kv PSUM -> bf16 block-diag kv tiles + per-head-replicated
                ksum tiles (so the den matmul broadcasts den across each
                head's 64 columns for free)."""
                for ct in range(NCT):
                    bank = kvacc[ct // 2]
                    base = (ct % 2) * 2 * HE
                    b_ = kvbpool.tile([P, P], BF16, tag=f"kvb{ct}",
                                      name=f"kvb{ct}")
                    nc.vector.memset(b_[:], 0.0)
                    nc.vector.tensor_copy(
                        b_[0:HD, 0:HD], bank[0:HD, base:base + HD])
                    nc.vector.tensor_copy(
                        b_[HD:P, HD:P],
                        bank[HD:P, base + HE:base + HE + HD])
                    kvb.append(b_)
                    # ksum columns staged to SBUF, then replicated across
                    # the head's 64 columns (free-dim broadcast copy)
                    kcol = tmp.tile([P, 2], F32, tag="kcol", name="kcol")
                    nc.vector.tensor_copy(
                        kcol[0:HD, 0:1], bank[0:HD, base + HD:base + HD + 1])
                    nc.vector.tensor_copy(
                        kcol[HD:P, 1:2],
                        bank[HD:P, base + HE + HD:base + HE + HD + 1])
                    s_ = kvbpool.tile([P, P], BF16, tag=f"ksr{ct}",
                                      name=f"ksr{ct}")
                    nc.vector.memset(s_[:], 0.0)
                    nc.vector.tensor_copy(
                        s_[0:HD, 0:HD], kcol[0:HD, 0:1].to_broadcast((HD, HD)))
                    nc.vector.tensor_copy(
                        s_[HD:P, HD:P], kcol[HD:P, 1:2].to_broadcast((HD, HD)))
                    ksr.append(s_)

            def b_qproj(cj, xt):
                qps = []
                for ct in range(NCT):
                    ps = pps.tile([P, CHUNK], F32, tag="pps", name="qps")
                    for kt in range(NKT):
                        nc.tensor.matmul(
                            ps[:],
                            w_t["q"][kt][:, ct * P:(ct + 1) * P],
                            xt[kt][:],
                            start=(kt == 0), stop=(kt == NKT - 1))
                    qps.append(ps)
                return qps

            def b_qfm(cj, qps):
                qft = []
                for ct in range(NCT):
                    bcol = bq_sb[:, ct:ct + 1]
                    e = tmp.tile([P, CHUNK], BF16, tag="tmp", name="qe")
                    nc.scalar.activation(e[:], qps[ct][:], Act.Exp, bias=bcol)
                    # relu on DVE here: pass B's ACT budget goes to ln/exp
                    r = tmp.tile([P, CHUNK], BF16, tag="tmp", name="qr")
                    nc.vector.tensor_scalar(
                        r[:], qps[ct][:], bcol, 0.0, Alu.add, Alu.max)
                    m = tmp.tile([P, CHUNK], BF16, tag="tmp", name="qm")
                    nc.vector.tensor_scalar(m[:], e[:], 1.0, None, Alu.min)
                    qf = qftpool.tile([P, CHUNK], BF16, tag="qft", name="qft")
                    nc.vector.tensor_tensor(qf[:], m[:], r[:], Alu.add)
                    qft.append(qf)
                qft_c[cj] = qft

            def b_dn(cj):
                """den/num matmuls for chunk cj (PE, [tokens, cols] layout).

                num_sub[s, e] accumulates per-ct 128-col blocks into one bank;
                dent[s, sub*8 + h] gets the per-head denominators. start=True
                only on each bank's first matmul (it zeroes the whole bank),
                the rest accumulate onto zeros in disjoint column ranges.
                """
                qft = qft_c[cj]
                dens, nums = [], []
                for sub in range(NSUB):
                    sl = slice(sub * P, (sub + 1) * P)
                    dps = dnps.tile([P, CHUNK], F32, tag="drep", name="dps")
                    for ct in range(NCT):
                        nc.tensor.matmul(
                            dps[:, ct * P:(ct + 1) * P],
                            qft[ct][:, sl], ksr[ct][:],
                            start=(ct == 0), stop=(ct == NCT - 1),
                            skip_group_check=True)
                    dens.append(dps)
                    nps = dnps.tile([P, CHUNK], F32, tag="num", name="nps")
                    for ct in range(NCT):
                        nc.tensor.matmul(
                            nps[:, ct * P:(ct + 1) * P],
                            qft[ct][:, sl], kvb[ct][:],
                            start=(ct == 0), stop=(ct == NCT - 1),
                            skip_group_check=True)
                    nums.append(nps)
                return dens, nums

            def b_div(cj, dn):
                """1/den = exp(-ln(den)) on ACT, multiply-evict on DVE."""
                tok0 = cj * CHUNK
                qft_c.pop(cj)
                dens, nums = dn
                for sub in range(NSUB):
                    u = tmp.tile([P, CHUNK], F32, tag="lden", name="lden")
                    nc.scalar.activation(u[:], dens[sub][:], Act.Ln)
                    rr = recpool.tile([P, CHUNK], BF16, tag="rec", name="rec")
                    nc.scalar.activation(rr[:], u[:], Act.Exp, scale=-1.0)
                    osb = outpool.tile([P, CG], BF16, tag="out", name="osb")
                    nc.vector.tensor_tensor(
                        osb[:], nums[sub][:], rr[:], Alu.mult)
                    nc.sync.dma_start(
                        out_d[tok0 + sub * P:tok0 + (sub + 1) * P, :],
                        osb[:])

            # ---- pass A ----
            xt_cur = xt0
            for ci in range(NCHUNK):
                a_chunk(ci, xt_cur)
                xt_cur = load_xt(ci + 1) if ci + 1 < NCHUNK else None
                if ci >= 1:
                    a_kv(ci - 1)
            xtb = load_xt(0)
            a_kv(NCHUNK - 1)

            # ---- pass B ----
            qps = b_qproj(0, xtb)
            bridge()
            b_qfm(0, qps)
            xtb = load_xt(1)
            dn_prev = None
            for cj in range(1, NCHUNK):
                qps = b_qproj(cj, xtb)
                xtb = load_xt(cj + 1) if cj + 1 < NCHUNK else None
                dn_prev = b_dn(cj - 1)
                b_div(cj - 1, dn_prev)
                b_qfm(cj, qps)
            dn_prev = b_dn(NCHUNK - 1)
            b_div(NCHUNK - 1, dn_prev)

    nc.compile()
    return nc


def _get_nc():
    global _CACHED_NC
    if _CACHED_NC is None:
        _CACHED_NC = _build()
    return _CACHED_NC


def _make_in_maps(hidden_states, Wq, bq, Wk, bk, Wv, bv):
    import ml_dtypes

    BF = ml_dtypes.bfloat16
    hs = np.asarray(hidden_states, np.float32)
    wq = np.asarray(Wq, np.float32)
    wk = np.asarray(Wk, np.float32)
    wv = np.asarray(Wv, np.float32)
    bq_ = np.asarray(bq, np.float32)
    bk_ = np.asarray(bk, np.float32)
    bv_ = np.asarray(bv, np.float32)
    xts = [np.ascontiguousarray(hs[b].T).astype(BF) for b in range(B)]
    in_maps = []
    for c in range(NCORES):
        b, g = divmod(c, 2)
        sl = slice(g * CG, (g + 1) * CG)
        in_maps.append({
            "xt": xts[b],
            "wq": wq[:, sl].astype(BF),
            "wk": wk[:, sl].astype(BF),
            "wv": wv[:, sl].astype(BF),
            "bq": np.ascontiguousarray(bq_[sl].reshape(NCT, P).T),
            "bk": np.ascontiguousarray(
                np.broadcast_to(bk_[sl], (P, CG))),
            "bv": np.ascontiguousarray(
                np.broadcast_to(bv_[sl], (P, CG))),
        })
    return in_maps


def _run(in_maps, **kwargs):
    from concourse.bass_utils import run_bass_kernel_spmd
    nc = _get_nc()
    return run_bass_kernel_spmd(nc, in_maps, core_ids=list(range(NCORES)),
                                **kwargs)


def _assemble(results):
    out = np.empty((B, S, D), np.float32)
    for c in range(NCORES):
        b, g = divmod(c, 2)
        out[b, :, g * CG:(g + 1) * CG] = results[c]["out"].astype(np.float32)
    return out


def kernel(hidden_states, Wq, bq, Wk, bk, Wv, bv):
    in_maps = _make_in_maps(hidden_states, Wq, bq, Wk, bk, Wv, bv)
    res = _run(in_maps)
    return _assemble(res.results)


# revision 21
# speedup vs baseline: 1.2954x; 1.2954x over previous
"""Trainium2 Bass kernel for BertLinearSelfAttention (linear attention).

Reference computation (per batch b, head h):
    q,k,v = X @ W{q,k,v} + b{q,k,v}            # [S, D] -> heads of 64
    qf, kf = elu(q)+1, elu(k)+1                # = min(exp(x),1) + max(x,0)
    kv[d,e]  = sum_s kf[s,d] v[s,e]            # [64, 64]
    ksum[d]  = sum_s kf[s,d]
    out[s,e] = (sum_d qf[s,d] kv[d,e]) / (sum_d qf[s,d] ksum[d])

Sharding: 8 cores = (4 batches) x (2 head-groups of 8 heads / 512 proj cols).
All matmul operands are bf16 (converted host-side), which keeps every matmul
at the PE's 1 row/cycle stream rate: fp32 operands are SBUF-read-bandwidth
bound (~0.92 ns/row measured vs 0.42 compute), bf16 halves the traffic.

Pass A (per 512-token chunk): k/v projections (tokens on partitions) +
feature maps on DVE/ACT + per-head kv/ksum accumulated directly in PSUM
across all chunks (ones-column in V' produces ksum).
Pass B (per chunk): q^T projection (cols on partitions), then per head-pair
block-diagonal den/num matmuls with N=512 moving tokens:
    den^T[p,s] = sum_k ksumrep[k,p] qf^T[k,s]   (ksum replicated across the
                 64 e-columns of its head, so the PE broadcasts den for free)
    num^T[e,s] = sum_d kv[d,e] qf^T[d,s]
Divide runs on DVE as the PSUM evict (recip + mult), output is stored
transposed [cols, tokens] in bf16 and re-transposed/upcast on the host.
"""

import os
import sys

import numpy as np

_REPO = "/opt/trn_rl_repo"
if os.path.isdir(_REPO) and _REPO not in sys.path:
    sys.path.insert(0, _REPO)

B, S, D, H, HD = 4, 4096, 1024, 16, 64
NCORES = 8
CG = 512            # projection columns per core (8 heads)
NH = CG // HD       # 8 heads per core
NCT = CG // 128     # 4 head-pair column tiles
HE = HD + 2         # vp cols per head: 64 v + 1 ones (ksum) + 1 pad
CHUNK = 512         # tokens per chunk
NSUB = CHUNK // 128     # 4 token sub-tiles per chunk
NCHUNK = S // CHUNK     # 8 chunks
NKT = D // 128          # 8 contraction tiles
P = 128

_CACHED_NC = None


def _build():
    import concourse.tile as tile
    from concourse import bacc, mybir
    from contextlib import ExitStack

    F32 = mybir.dt.float32
    BF16 = mybir.dt.bfloat16
    Alu = mybir.AluOpType
    Act = mybir.ActivationFunctionType

    nc = bacc.Bacc("TRN2", target_bir_lowering=False, debug=False,
                   num_devices=NCORES)

    xt_d = nc.dram_tensor("xt", [D, S], BF16, kind="ExternalInput").ap()
    w_d = {
        "q": nc.dram_tensor("wq", [D, CG], BF16, kind="ExternalInput").ap(),
        "k": nc.dram_tensor("wk", [D, CG], BF16, kind="ExternalInput").ap(),
        "v": nc.dram_tensor("wv", [D, CG], BF16, kind="ExternalInput").ap(),
    }
    bq_d = nc.dram_tensor("bq", [P, NCT], F32, kind="ExternalInput").ap()
    # bk/bv pre-replicated across partitions host-side (keeps gpsimd as a
    # pure DMA queue — its compute path costs a ~12us LOAD_LIB+DRAIN)
    bk_d = nc.dram_tensor("bk", [P, CG], F32, kind="ExternalInput").ap()
    bv_d = nc.dram_tensor("bv", [P, CG], F32, kind="ExternalInput").ap()
    out_d = nc.dram_tensor("out", [S, CG], BF16, kind="ExternalOutput").ap()

    with tile.TileContext(nc) as tc:
        with ExitStack() as ctx:
            const = ctx.enter_context(tc.tile_pool(name="const", bufs=1))
            wpool = ctx.enter_context(tc.tile_pool(name="wpool", bufs=1))
            xtpool = ctx.enter_context(tc.tile_pool(name="xtpool", bufs=24))
            kfpool = ctx.enter_context(tc.tile_pool(name="kfpool", bufs=8))
            vppool = ctx.enter_context(tc.tile_pool(name="vppool", bufs=8))
            qftpool = ctx.enter_context(tc.tile_pool(name="qftpool", bufs=8))
            tmp = ctx.enter_context(tc.tile_pool(name="tmp", bufs=10))
            outpool = ctx.enter_context(tc.tile_pool(name="outp", bufs=6))
            recpool = ctx.enter_context(tc.tile_pool(name="recp", bufs=8))
            kvbpool = ctx.enter_context(tc.tile_pool(name="kvbp", bufs=1))
            pps = ctx.enter_context(
                tc.tile_pool(name="pps", bufs=2, space="PSUM"))
            kvps = ctx.enter_context(
                tc.tile_pool(name="kvps", bufs=1, space="PSUM"))
            dnps = ctx.enter_context(
                tc.tile_pool(name="dnps", bufs=2, space="PSUM"))

            def load_xt(ci):
                tok0 = ci * CHUNK
                xt = []
                for kt in range(NKT):
                    t = xtpool.tile([P, CHUNK], BF16, tag="xt", name="xt")
                    nc.sync.dma_start(
                        t[:], xt_d[kt * P:(kt + 1) * P, tok0:tok0 + CHUNK])
                    xt.append(t)
                return xt

            # queue the first chunk's X^T ahead of all setup DMAs
            xt0 = load_xt(0)

            # ---- weights (per-kt tiles so the first matmul only waits on
            # its own 128KB slice), k/v interleaved first, q later ----
            w_t = {"q": [], "k": [], "v": []}
            for kt in range(NKT):
                for nm in ("k", "v"):
                    t = wpool.tile([P, CG], BF16, tag=f"w{nm}{kt}",
                                   name=f"w{nm}{kt}")
                    nc.gpsimd.dma_start(t[:], w_d[nm][kt * P:(kt + 1) * P, :])
                    w_t[nm].append(t)
            for kt in range(NKT):
                t = wpool.tile([P, CG], BF16, tag=f"wq{kt}", name=f"wq{kt}")
                nc.gpsimd.dma_start(t[:], w_d["q"][kt * P:(kt + 1) * P, :])
                w_t["q"].append(t)

            # ---- constants ----
            bq_sb = const.tile([P, NCT], F32, tag="bq", name="bq_sb")
            nc.gpsimd.dma_start(bq_sb[:], bq_d[:])
            bk_rep = const.tile([P, CG], F32, tag="bkrep", name="bk_rep")
            nc.sync.dma_start(bk_rep[:], bk_d[:])
            bv_rep = const.tile([P, CG], F32, tag="bvrep", name="bv_rep")
            nc.sync.dma_start(bv_rep[:], bv_d[:])
            # tail columns for V': [1.0 (ksum), 0.0 (pad)] per head
            ones_tail = const.tile([P, NH * 2], BF16, tag="otail",
                                   name="ones_tail")
            nc.vector.memset(ones_tail[:], 0.0)
            nc.vector.memset(
                ones_tail[:].rearrange("p (h e) -> p h e", e=2)[:, :, 0:1],
                1.0)

            # kv/ksum accumulators: bank i holds ct=2i (cols 0:132) and
            # ct=2i+1 (cols 132:264); within a ct: even head on partitions
            # 0:64 cols 0:66, odd head on partitions 64:128 cols 66:132.
            # NOTE: matmul start=True zeroes the full bank width for the
            # partitions it writes, so concurrent accumulation groups in one
            # bank must NOT use start; memset once and accumulate throughout.
            kvacc = [kvps.tile([P, 4 * HE], F32, tag=f"kvacc{i}",
                               name=f"kvacc{i}") for i in range(2)]
            for i in range(2):
                nc.vector.memset(kvacc[i][:], 0.0)

            kf_c = {}
            vp_c = {}
            qft_c = {}

            def a_chunk(ci, xt):
                kfs, vps = [], []
                for sub in range(NSUB):
                    sl = slice(sub * P, (sub + 1) * P)
                    kps = pps.tile([P, CG], F32, tag="pps", name="kps")
                    for kt in range(NKT):
                        nc.tensor.matmul(
                            kps[:], xt[kt][:, sl], w_t["k"][kt][:],
                            start=(kt == 0), stop=(kt == NKT - 1))
                    vps_ = pps.tile([P, CG], F32, tag="pps", name="vps")
                    for kt in range(NKT):
                        nc.tensor.matmul(
                            vps_[:], xt[kt][:, sl], w_t["v"][kt][:],
                            start=(kt == 0), stop=(kt == NKT - 1))
                    # k feature map: kf = min(exp(k+bk),1) + max(k+bk,0)
                    kb = tmp.tile([P, CG], BF16, tag="tmp", name="kb")
                    nc.vector.tensor_tensor(kb[:], kps[:], bk_rep[:], Alu.add)
                    e = tmp.tile([P, CG], BF16, tag="tmp", name="e")
                    nc.scalar.activation(e[:], kb[:], Act.Exp)
                    r = tmp.tile([P, CG], BF16, tag="tmp", name="r")
                    nc.scalar.activation(r[:], kb[:], Act.Relu)
                    m = tmp.tile([P, CG], BF16, tag="tmp", name="m")
                    nc.vector.tensor_scalar(m[:], e[:], 1.0, None, Alu.min)
                    kf = kfpool.tile([P, CG], BF16, tag="kf", name="kf")
                    nc.vector.tensor_tensor(kf[:], m[:], r[:], Alu.add)
                    kfs.append(kf)
                    # V' = [v + bv | 1 | 0] per head
                    vp = vppool.tile([P, NH * HE], BF16, tag="vp", name="vp")
                    nc.vector.tensor_tensor(
                        vp[:].rearrange("p (h e) -> p h e", e=HE)[:, :, :HD],
                        vps_[:].rearrange("p (h e) -> p h e", e=HD),
                        bv_rep[:].rearrange("p (h e) -> p h e", e=HD),
                        Alu.add)
                    nc.vector.tensor_copy(
                        vp[:].rearrange("p (h e) -> p h e", e=HE)[:, :, HD:],
                        ones_tail[:].rearrange("p (h e) -> p h e", e=2))
                    vps.append(vp)
                kf_c[ci] = kfs
                vp_c[ci] = vps

            def a_kv(ci):
                kfs, vps = kf_c.pop(ci), vp_c.pop(ci)
                for ct in range(NCT):
                    bank = kvacc[ct // 2]
                    base = (ct % 2) * 2 * HE
                    for par in range(2):
                        h = 2 * ct + par
                        dst = bank[par * HD:(par + 1) * HD,
                                   base + par * HE:base + (par + 1) * HE]
                        for sub in range(NSUB):
                            nc.tensor.matmul(
                                dst,
                                kfs[sub][:, h * HD:(h + 1) * HD],
                                vps[sub][:, h * HE:(h + 1) * HE],
                                start=False,
                                stop=(ci == NCHUNK - 1 and sub == NSUB - 1),
                                skip_group_check=True)

            kvb = []
            ksr = []

            def bridge():
                """kv PSUM -> bf16 block-diag kv tiles + per-head-replicated
                ksum tiles (so the den matmul broadcasts den across each
                head's 64 columns for free)."""
                for ct in range(NCT):
                    bank = kvacc[ct // 2]
                    base = (ct % 2) * 2 * HE
                    b_ = kvbpool.tile([P, P], BF16, tag=f"kvb{ct}",
                                      name=f"kvb{ct}")
                    nc.vector.memset(b_[:], 0.0)
                    nc.vector.tensor_copy(
                        b_[0:HD, 0:HD], bank[0:HD, base:base + HD])
                    nc.vector.tensor_copy(
                        b_[HD:P, HD:P],
                        bank[HD:P, base + HE:base + HE + HD])
                    kvb.append(b_)
                    # ksum columns: [ksum_even | 0 ; 0 | ksum_odd]
                    s_ = kvbpool.tile([P, 2], BF16, tag=f"ks2{ct}",
                                      name=f"ks2{ct}")
                    nc.vector.memset(s_[:], 0.0)
                    nc.vector.tensor_copy(
                        s_[0:HD, 0:1], bank[0:HD, base + HD:base + HD + 1])
                    nc.vector.tensor_copy(
                        s_[HD:P, 1:2],
                        bank[HD:P, base + HE + HD:base + HE + HD + 1])
                    ksr.append(s_)

            def b_qproj(cj, xt):
                qps = []
                for ct in range(NCT):
                    ps = pps.tile([P, CHUNK], F32, tag="pps", name="qps")
                    for kt in range(NKT):
                        nc.tensor.matmul(
                            ps[:],
                            w_t["q"][kt][:, ct * P:(ct + 1) * P],
                            xt[kt][:],
                            start=(kt == 0), stop=(kt == NKT - 1))
                    qps.append(ps)
                return qps

            def b_qfm(cj, qps):
                qft = []
                for ct in range(NCT):
                    bcol = bq_sb[:, ct:ct + 1]
                    e = tmp.tile([P, CHUNK], BF16, tag="tmp", name="qe")
                    nc.scalar.activation(e[:], qps[ct][:], Act.Exp, bias=bcol)
                    r = tmp.tile([P, CHUNK], BF16, tag="tmp", name="qr")
                    nc.scalar.activation(r[:], qps[ct][:], Act.Relu, bias=bcol)
                    m = tmp.tile([P, CHUNK], BF16, tag="tmp", name="qm")
                    nc.vector.tensor_scalar(m[:], e[:], 1.0, None, Alu.min)
                    qf = qftpool.tile([P, CHUNK], BF16, tag="qft", name="qft")
                    nc.vector.tensor_tensor(qf[:], m[:], r[:], Alu.add)
                    qft.append(qf)
                qft_c[cj] = qft

            def b_dn(cj):
                """den/num matmuls for chunk cj (PE, [tokens, cols] layout).

                num_sub[s, e] accumulates per-ct 128-col blocks into one bank;
                dent[s, sub*8 + h] gets the per-head denominators. start=True
                only on each bank's first matmul (it zeroes the whole bank),
                the rest accumulate onto zeros in disjoint column ranges.
                """
                qft = qft_c[cj]
                dent = dnps.tile([P, NSUB * NH], F32, tag="dent", name="dent")
                nums = []
                for sub in range(NSUB):
                    sl = slice(sub * P, (sub + 1) * P)
                    for ct in range(NCT):
                        nc.tensor.matmul(
                            dent[:, sub * NH + 2 * ct:sub * NH + 2 * ct + 2],
                            qft[ct][:, sl], ksr[ct][:],
                            start=(sub == 0 and ct == 0),
                            stop=(sub == NSUB - 1 and ct == NCT - 1),
                            skip_group_check=True)
                    nps = dnps.tile([P, CHUNK], F32, tag="num", name="nps")
                    for ct in range(NCT):
                        nc.tensor.matmul(
                            nps[:, ct * P:(ct + 1) * P],
                            qft[ct][:, sl], kvb[ct][:],
                            start=(ct == 0), stop=(ct == NCT - 1),
                            skip_group_check=True)
                    nums.append(nps)
                return dent, nums

            def b_div(cj, dn):
                """small reciprocal + broadcast-copy + multiply-evict (DVE)."""
                tok0 = cj * CHUNK
                qft_c.pop(cj)
                dent, nums = dn
                for sub in range(NSUB):
                    rec = recpool.tile([P, NH], F32, tag="rec", name="rec")
                    nc.vector.reciprocal(
                        rec[:], dent[:, sub * NH:(sub + 1) * NH])
                    rep = recpool.tile([P, CG], F32, tag="rep", name="rep")
                    nc.vector.tensor_copy(
                        rep[:].rearrange("p (h e) -> p h e", e=HD),
                        rec[:].rearrange("p (h e) -> p h e", e=1)
                        .to_broadcast((P, NH, HD)))
                    osb = outpool.tile([P, CG], BF16, tag="out", name="osb")
                    nc.vector.tensor_tensor(
                        osb[:], nums[sub][:], rep[:], Alu.mult)
                    nc.sync.dma_start(
                        out_d[tok0 + sub * P:tok0 + (sub + 1) * P, :],
                        osb[:])

            # ---- pass A ----
            xt_cur = xt0
            for ci in range(NCHUNK):
                a_chunk(ci, xt_cur)
                xt_cur = load_xt(ci + 1) if ci + 1 < NCHUNK else None
                if ci >= 1:
                    a_kv(ci - 1)
            xtb = load_xt(0)
            a_kv(NCHUNK - 1)

            # ---- pass B ----
            qps = b_qproj(0, xtb)
            bridge()
            b_qfm(0, qps)
            xtb = load_xt(1)
            dn_prev = None
            for cj in range(1, NCHUNK):
                qps = b_qproj(cj, xtb)
                xtb = load_xt(cj + 1) if cj + 1 < NCHUNK else None
                dn_prev = b_dn(cj - 1)
                b_div(cj - 1, dn_prev)
                b_qfm(cj, qps)
            dn_prev = b_dn(NCHUNK - 1)
            b_div(NCHUNK - 1, dn_prev)

    nc.compile()
    return nc


def _get_nc():
    global _CACHED_NC
    if _CACHED_NC is None:
        _CACHED_NC = _build()
    return _CACHED_NC


def _make_in_maps(hidden_states, Wq, bq, Wk, bk, Wv, bv):
    import ml_dtypes

    BF = ml_dtypes.bfloat16
    hs = np.asarray(hidden_states, np.float32)
    wq = np.asarray(Wq, np.float32)
    wk = np.asarray(Wk, np.float32)
    wv = np.asarray(Wv, np.float32)
    bq_ = np.asarray(bq, np.float32)
    bk_ = np.asarray(bk, np.float32)
    bv_ = np.asarray(bv, np.float32)
    xts = [np.ascontiguousarray(hs[b].T).astype(BF) for b in range(B)]
    in_maps = []
    for c in range(NCORES):
        b, g = divmod(c, 2)
        sl = slice(g * CG, (g + 1) * CG)
        in_maps.append({
            "xt": xts[b],
            "wq": wq[:, sl].astype(BF),
            "wk": wk[:, sl].astype(BF),
            "wv": wv[:, sl].astype(BF),
            "bq": np.ascontiguousarray(bq_[sl].reshape(NCT, P).T),
            "bk": np.ascontiguousarray(
                np.broadcast_to(bk_[sl], (P, CG))),
            "bv": np.ascontiguousarray(
                np.broadcast_to(bv_[sl], (P, CG))),
        })
    return in_maps


def _run(in_maps, **kwargs):
    from concourse.bass_utils import run_bass_kernel_spmd
    nc = _get_nc()
    return run_bass_kernel_spmd(nc, in_maps, core_ids=list(range(NCORES)),
                                **kwargs)


def _assemble(results):
    out = np.empty((B, S, D), np.float32)
    for c in range(NCORES):
        b, g = divmod(c, 2)
        out[b, :, g * CG:(g + 1) * CG] = results[c]["out"].astype(np.float32)
    return out


def kernel(hidden_states, Wq, bq, Wk, bk, Wv, bv):
    in_maps = _make_in_maps(hidden_states, Wq, bq, Wk, bk, Wv, bv)
    res = _run(in_maps)
    return _assemble(res.results)
